# revision 1
# baseline (speedup 1.0000x reference)
"""Trainium2 Bass kernel for an autoregressive-flow (MAF) layer.

Reference computation (per region r, batch-network b):
    xr[n, d]   = x[n, region_idx[r, d]]                      # [N, D]
    h1 = relu(xr @ (W1*M1)[r,b])                             # [N, H]
    h2 = relu(h1 @ (W2*M2)[r,b])                             # [N, H]
    o  = h2 @ (W3*M3)[r,b]                                   # [N, 2D]
    shift = o[:, 0::2]; log_scale = o[:, 1::2]
    u  = (xr - shift) * exp(-log_scale)
    ll[n, r, b] = sum_d(-0.5*u^2 - 0.5*log(2*pi) - log_scale)

Sharding: region axis R=8 across the 8 NeuronCores; each core handles its
region's B=16 networks over all N=2048 samples.

Device dataflow (per core, "transposed" orientation), v2:
    - xtb [128, 2048] bf16: x-slice transposed, replicated on 4 partition
      row-groups (feeds 4x row-packed K=32 L1 matmuls + the seed matmul).
    - All weights+masks arrive packed in one bf16 "wall" tensor so input
      DMA is 2 triggers for weights (group-0 first) + 2 for x; masked
      weights are built once on DVE (bf16 2x mode) and stay resident.
    - Per (chunk, group) work item: L1 (4 row-packed K=32 matmuls),
      relu-moves to bf16 SBUF split across ACT/DVE/Pool, L2 (4 full
      matmuls), relu-moves, L3 split into shift/log_scale halves with 4
      networks column-packed per PSUM tile; shift tile seeded with -x via
      a negated tiled-identity matmul so PSUM holds (shift - x).
    - Tail: A = 0.5*(shift-x)^2 [ACT Square, scale=sqrt(0.5), bf16],
      B = exp(-2*ls) [ACT Exp, bf16], l_sb = copy(ls) [Pool, bf16],
      c = A*B [DVE 2x bf16], v = c + l_sb [DVE 2x bf16 -> fp32r].
      One ll matmul per group (-1 block weights, fp32r) accumulates
      -(0.5*u^2 + ls) into a [16, 512] PSUM tile; bias add on DVE.
    - PE warmup: ~10 short matmuls right after the first (small) DMA lands
      keep the HAM activity window busy so the PE clock is at 2.4 GHz
      (K=8/8) by the time real matmuls issue; emission of L3/ll is
      staggered one work item behind L1/L2 so the PE FIFO never
      head-of-line blocks on the relu-move/tail chains.
"""

import ml_dtypes
import numpy as np

import concourse.bacc as bacc
import concourse.mybir as mybir
from concourse.bass_utils import run_bass_kernel_spmd
from concourse.tile import TileContext

R, B, D, H, N, F = 8, 16, 32, 128, 2048, 256
HALF_LOG_2PI = 0.9189385332046727
N_CORES = 8
CHUNK = 512
F32 = mybir.dt.float32
F32R = mybir.dt.float32r
BF16 = mybir.dt.bfloat16

# wall column layout (bf16): [neg(128) | 4 x group-block(1792)]
#   group-block: w1w(128) w1m(128) w2w(512) w2m(512) w3w(256) w3m(256)
GBLK = 1792
WALL_C = 128 + 4 * GBLK
SQRT_HALF = float(np.sqrt(0.5))


def _neg_block():
    # Negated tiled identity: lhsT [32, 128], out rows 32*bp + d get -x_d.
    neg = np.zeros((128, 128), np.float32)
    for m in range(128):
        neg[m % D, m] = -1.0
    return neg


def _llw():
    # ll weights [128, 4, 16] fp32: for group g, col j = 4g+bp sums rows
    # 32bp..32bp+31 with -1 (v already holds 0.5*u^2 + ls).
    w = np.zeros((128, 4, 16), np.float32)
    for g in range(4):
        for bp in range(4):
            w[32 * bp : 32 * (bp + 1), g, 4 * g + bp] = -1.0
    return w.reshape(128, 64)


def build_nc(n_total=N):
    assert n_total % CHUNK == 0
    n_chunks = n_total // CHUNK
    n_items = 4 * n_chunks  # (chunk, group) work items

    nc = bacc.Bacc(
        "TRN2",
        target_bir_lowering=False,
        debug=False,
        enable_asserts=False,
        num_devices=N_CORES,
    )

    wall_d = nc.declare_dram_parameter("wall", [128, WALL_C], BF16, isOutput=False)
    llw_d = nc.declare_dram_parameter("llw", [128, 64], F32, isOutput=False)
    xt4_d = nc.declare_dram_parameter("xt4", [128, n_total], BF16, isOutput=False)
    out_d = nc.declare_dram_parameter("out", [n_chunks, 16, CHUNK], F32, isOutput=True)

    with TileContext(nc) as tc:
        with (
            tc.tile_pool(name="const", bufs=1) as cpool,
            tc.tile_pool(name="stage", bufs=1) as spool,
            tc.tile_pool(name="s1", bufs=8) as s1pool,
            tc.tile_pool(name="s2", bufs=8) as s2pool,
            tc.tile_pool(name="tail", bufs=2) as tpool,
            tc.tile_pool(name="vout", bufs=2) as vpool,
            tc.tile_pool(name="llo", bufs=2) as opool,
            tc.tile_pool(name="p1", bufs=3, space="PSUM") as p1pool,
            tc.tile_pool(name="p2", bufs=2, space="PSUM") as p2pool,
            tc.tile_pool(name="pt", bufs=1, space="PSUM") as ptpool,
            tc.tile_pool(name="pl", bufs=1, space="PSUM") as plpool,
            tc.tile_pool(name="pll", bufs=1, space="PSUM") as pllpool,
        ):
            wall = cpool.tile([128, WALL_C], BF16, tag="wall")
            llwst = spool.tile([128, 64], F32, tag="llwst")
            llwr = cpool.tile([128, 64], F32R, tag="llwr")
            xtb = cpool.tile([128, n_total], BF16, tag="xtb")
            w1m = cpool.tile([128, 512], BF16, tag="w1m")
            w2m = cpool.tile([128, 2048], BF16, tag="w2m")
            w3m = cpool.tile([128, 1024], BF16, tag="w3m")

            # DMA order: consts+g0 weights first, then x chunk 0, the rest
            # of the weights, then x chunks 1-3.
            nc.sync.dma_start(out=wall[:, : 128 + GBLK], in_=wall_d[:, : 128 + GBLK])
            nc.sync.dma_start(out=llwst[:], in_=llw_d[:])
            nc.sync.dma_start(out=xtb[:, :CHUNK], in_=xt4_d[:, :CHUNK])
            nc.sync.dma_start(
                out=wall[:, 128 + GBLK :], in_=wall_d[:, 128 + GBLK :]
            )
            nc.sync.dma_start(out=xtb[:, CHUNK:], in_=xt4_d[:, CHUNK:])

            neg = wall[0:D, 0:128]

            nc.vector.tensor_copy(out=llwr[:], in_=llwst[:])

            # Masked weights on DVE (bf16 2x mode). Group 0 is built up
            # front; groups 1-3 are emitted inside the loop (end of slot
            # g-1) so the DVE FIFO never head-of-line blocks on the big
            # weight DMA while slot-0 relu moves are ready.
            def emit_wmask(g):
                base = 128 + g * GBLK
                nc.gpsimd.tensor_mul(
                    out=w1m[:, 128 * g : 128 * (g + 1)],
                    in0=wall[:, base : base + 128],
                    in1=wall[:, base + 128 : base + 256],
                )
                nc.gpsimd.tensor_mul(
                    out=w2m[:, 512 * g : 512 * (g + 1)],
                    in0=wall[:, base + 256 : base + 768],
                    in1=wall[:, base + 768 : base + 1280],
                )
                nc.gpsimd.tensor_mul(
                    out=w3m[:, 256 * g : 256 * (g + 1)],
                    in0=wall[:, base + 1280 : base + 1536],
                    in1=wall[:, base + 1536 : base + 1792],
                )

            emit_wmask(0)

            # Per-item state carried between staggered emission phases.
            st = [None] * n_items

            def item_cg(i):
                return i // 4, i % 4

            def emit_L1(i, bps):
                c, g = item_cg(i)
                cs = slice(c * CHUNK, (c + 1) * CHUNK)
                s = st[i]
                for bp in bps:
                    prow = slice(32 * bp, 32 * (bp + 1))
                    p1 = p1pool.tile([128, CHUNK], F32, tag="p1")
                    nc.tensor.matmul(
                        p1[:],
                        w1m[prow, 128 * g : 128 * (g + 1)],
                        xtb[prow, cs],
                        start=True,
                        stop=True,
                        tile_position=(32 * bp, 0),
                    )
                    s["p1"][bp] = p1

            def emit_s1_moves(i):
                # bp -> engine: [ACT, DVE, ACT, DVE] (GpSimd cannot read PSUM)
                s = st[i]
                for bp in range(4):
                    s1 = s1pool.tile([128, CHUNK], BF16, tag="s1")
                    p1 = s["p1"][bp]
                    if bp in (0, 2):
                        nc.scalar.activation(
                            s1[:], p1[:], mybir.ActivationFunctionType.Relu
                        )
                    else:
                        nc.vector.tensor_scalar_max(s1[:], p1[:], 0.0)
                    s["s1"][bp] = s1

            def emit_L2(i):
                c, g = item_cg(i)
                s = st[i]
                for bp in range(4):
                    b = 4 * g + bp
                    p2 = p2pool.tile([128, CHUNK], F32, tag="p2")
                    nc.tensor.matmul(
                        p2[:],
                        w2m[:, 128 * b : 128 * (b + 1)],
                        s["s1"][bp][:],
                        start=True,
                        stop=True,
                    )
                    s["p2"][bp] = p2

            def emit_s2_moves(i):
                # bp -> engine: [DVE, ACT, DVE, ACT]
                s = st[i]
                for bp in range(4):
                    s2 = s2pool.tile([128, CHUNK], BF16, tag="s2")
                    p2 = s["p2"][bp]
                    if bp in (1, 3):
                        nc.scalar.activation(
                            s2[:], p2[:], mybir.ActivationFunctionType.Relu
                        )
                    else:
                        nc.vector.tensor_scalar_max(s2[:], p2[:], 0.0)
                    s["s2"][bp] = s2

            def emit_L3_shift(i):
                c, g = item_cg(i)
                cs = slice(c * CHUNK, (c + 1) * CHUNK)
                s = st[i]
                tps = ptpool.tile([128, CHUNK], F32, tag="tps")
                nc.tensor.matmul(
                    tps[:],
                    neg,
                    xtb[0:D, cs],
                    start=True,
                    stop=False,
                    skip_group_check=True,
                    tile_position=(0, 0),
                )
                for bp in range(4):
                    b = 4 * g + bp
                    nc.tensor.matmul(
                        tps[32 * bp : 32 * (bp + 1), :],
                        w3m[:, 64 * b : 64 * b + 32],
                        s["s2"][bp][:],
                        start=False,
                        stop=(bp == 3),
                        skip_group_check=True,
                        tile_position=(0, 32 * bp),
                    )
                s["tps"] = tps

            def emit_L3_ls(i, bps):
                c, g = item_cg(i)
                s = st[i]
                if s.get("lps") is None:
                    s["lps"] = plpool.tile([128, CHUNK], F32, name="lps", tag="lps")
                lps = s["lps"]
                for bp in bps:
                    b = 4 * g + bp
                    nc.tensor.matmul(
                        lps[32 * bp : 32 * (bp + 1), :],
                        w3m[:, 64 * b + 32 : 64 * b + 64],
                        s["s2"][bp][:],
                        start=True,
                        stop=True,
                        tile_position=(0, 32 * bp),
                    )

            def emit_tail(i):
                s = st[i]
                a_sb = tpool.tile([128, CHUNK], BF16, tag="a")
                nc.scalar.activation(
                    a_sb[:],
                    s["tps"][:],
                    mybir.ActivationFunctionType.Square,
                    scale=SQRT_HALF,
                )
                b_sb = tpool.tile([128, CHUNK], BF16, tag="b")
                nc.scalar.activation(
                    b_sb[:],
                    s["lps"][:],
                    mybir.ActivationFunctionType.Exp,
                    scale=-2.0,
                )
                c_sb = tpool.tile([128, CHUNK], BF16, tag="c")
                nc.vector.tensor_mul(out=c_sb[:], in0=a_sb[:], in1=b_sb[:])
                v = vpool.tile([128, CHUNK], F32R, tag="v")
                nc.vector.tensor_add(out=v[:], in0=c_sb[:], in1=s["lps"][:])
                s["v"] = v

            llps_ref = [None]

            def emit_ll(i):
                c, g = item_cg(i)
                if g == 0:
                    llps_ref[0] = pllpool.tile([16, CHUNK], F32, name="llps", tag="llps")
                llps = llps_ref[0]
                nc.tensor.matmul(
                    llps[:],
                    llwr[:, 16 * g : 16 * (g + 1)],
                    st[i]["v"][:],
                    start=(g == 0),
                    stop=(g == 3),
                    skip_group_check=True,
                )
                if g == 3:
                    ll_sb = opool.tile([16, CHUNK], F32, tag="ll")
                    nc.vector.tensor_scalar_add(
                        ll_sb[:], llps[:], float(-D * HALF_LOG_2PI)
                    )
                    nc.sync.dma_start(out=out_d[c], in_=ll_sb[:])
                st[i]["v"] = None
                st[i] = None

            # Staggered emission: PE slot i runs item i's L3 + item (i-1)'s
            # ll interleaved with item (i+1)'s L1/L2, ordered so the PE FIFO
            # never head-of-line blocks on relu-move/tail chains or PSUM
            # bank reuse (p1/p2 are double-buffered; each matmul that reuses
            # a bank issues well after the relu move that frees it).
            def start_item(i):
                st[i] = {"p1": [None] * 4, "s1": [None] * 4,
                         "p2": [None] * 4, "s2": [None] * 4, "lps": None}

            start_item(0)
            emit_L1(0, range(4))
            emit_s1_moves(0)
            emit_L2(0)
            emit_s2_moves(0)
            for i in range(n_items):
                nxt = i + 1
                if nxt < n_items:
                    start_item(nxt)
                    if nxt <= 3:
                        emit_wmask(nxt)
                    emit_L1(nxt, [0, 1, 2])
                emit_L3_shift(i)
                if nxt < n_items:
                    emit_L1(nxt, [3])
                    emit_s1_moves(nxt)
                emit_L3_ls(i, [0, 1, 2, 3])
                emit_tail(i)
                if i >= 1:
                    emit_ll(i - 1)
                if nxt < n_items:
                    emit_L2(nxt)
                    emit_s2_moves(nxt)
            emit_ll(n_items - 1)

    nc.compile()
    return nc


def shard_inputs(x, W1, W2, W3, M1, M2, M3, region_idx, n_total=N):
    """Per-core input dicts: pure gather/transpose/replicate layout prep."""
    x = np.asarray(x, dtype=np.float32)
    region_idx = np.asarray(region_idx)
    neg = _neg_block()
    llw = _llw()
    in_maps = []
    for r in range(N_CORES):
        xr = x[:n_total, region_idx[r]]  # [n, D]
        xt = np.ascontiguousarray(xr.T)  # [D, n]
        xt4 = np.ascontiguousarray(np.tile(xt, (4, 1)))  # [128, n]

        wall = np.zeros((128, WALL_C), np.float32)
        wall[:, 0:128] = neg
        for g in range(4):
            base = 128 + g * GBLK

            def blk14(w):
                # [4, D, H] for nets 4g..4g+3 -> rows 32*bp + d
                return np.asarray(w[r], np.float32).reshape(4, 4, D, H)[g].reshape(
                    128, H
                )

            wall[:, base : base + 128] = blk14(W1)
            wall[:, base + 128 : base + 256] = blk14(M1)

            def blk2(w):
                # concat over bp of [H, H] (lhsT: partition = h_in)
                return np.concatenate(
                    [np.asarray(w[r][4 * g + bp], np.float32) for bp in range(4)],
                    axis=1,
                )

            wall[:, base + 256 : base + 768] = blk2(W2)
            wall[:, base + 768 : base + 1280] = blk2(M2)

            def blk3(w):
                # per net [H, 64] = [shift cols (0::2) | ls cols (1::2)]
                cols = []
                for bp in range(4):
                    wb = np.asarray(w[r][4 * g + bp], np.float32)  # [H, 2D]
                    cols.append(np.concatenate([wb[:, 0::2], wb[:, 1::2]], axis=1))
                return np.concatenate(cols, axis=1)

            wall[:, base + 1280 : base + 1536] = blk3(W3)
            wall[:, base + 1536 : base + 1792] = blk3(M3)

        in_maps.append(
            {
                "wall": wall.astype(ml_dtypes.bfloat16),
                "llw": llw,
                "xt4": xt4.astype(ml_dtypes.bfloat16),
            }
        )
    return in_maps


_NC_CACHE = {}


def run(x, W1, W2, W3, M1, M2, M3, region_idx, trace=False, n_total=N):
    if n_total not in _NC_CACHE:
        _NC_CACHE[n_total] = build_nc(n_total)
    nc = _NC_CACHE[n_total]
    in_maps = shard_inputs(x, W1, W2, W3, M1, M2, M3, region_idx, n_total)
    res = run_bass_kernel_spmd(
        nc, in_maps, core_ids=list(range(N_CORES)), trace=trace
    )
    out = np.empty((n_total, R, B), dtype=np.float32)
    for r in range(N_CORES):
        o = res.results[r]["out"]  # [n_chunks, 16, CHUNK]
        out[:, r, :] = o.transpose(0, 2, 1).reshape(n_total, B)
    return out, res


def kernel(x, W1, W2, W3, M1, M2, M3, region_idx):
    out, _ = run(x, W1, W2, W3, M1, M2, M3, region_idx)
    return out

